# revision 44
# baseline (speedup 1.0000x reference)
# Involution2d (K=7) Trainium2 kernel — 8-core SPMD, batch+spatial sharding.
#
# Sharding: 8 cores = (batch b in 0..3) x (H-half in 0..1). Each core computes
# a [128, 32, 64] output block. fp16 data path (rel-err budget 2e-2; lands
# ~2.4e-3).
#
# Per-core pipeline, software-pipelined across four engines:
#   1. f = relu(w1s^T @ x + b1f)            TensorE + ScalarE   [32, 2048]
#   2. per offset o (49 total):
#      bc_o  = W2BC_o @ f                   TensorE (K=32)      [128, 2048] PSUM
#        W2BC_o = w2 row o replicated into 128 columns (host-precomputed), so
#        the per-pixel kernel value is generated ALREADY BROADCAST over the
#        128 channel partitions; the [49, P] kernel map is never materialized
#        and there is no per-offset DMA.
#      bcs_o = bc_o + b2[o]                 ScalarE PSUM->SBUF fp16
#      prod  = bcs_o * x_shift(o)           VectorE fp16 2x_1P mode
#      acc  += prod                         VectorE fp16 2x_1P mode
#
# The 98-op VectorE stream (~1.14us per [128,2048] tensor_tensor) is the
# bottleneck and runs gap-free; TensorE (4 K=32 matmuls/offset) and ScalarE
# (PSUM evacuation + bias) hide underneath it.
#
# Layout tricks:
# - x lives in a guarded stride-70 row layout (3 zero cols per row edge,
#   3 halo rows top/bottom, zero-filled) so every shifted read is exact zero
#   padding — no W-edge mask, no wrap garbage. Shifted reads are 3D APs
#   [128, 32, 64] with row stride 70.
# - A twin copy of x offset by one element (built on-device by VectorE
#   during the DMA prologue) keeps odd-dj offsets 4-byte aligned — the DVE
#   2x_1P perf mode requires 4B-aligned stride-1 operands.
# - Offsets are processed xa-aligned-first so the twin is not needed until
#   ~60us into the stream.
# - acc/bcs/prod/f are dense stride-1 tiles; PSUM chunks are bank-aligned.
# - Input DMAs are split across the sync/scalar/gpsimd hardware queues and
#   ordered by when each consumer needs the data; the output is written in
#   thirds overlapped with the final accumulation.
#
# Rejected alternatives (measured): involution-as-banded-matmul (the skew/
# diagonal Kmat build has no fast primitive on TRN2), GpSimd elementwise
# offload (fp16 and fp32 both slower + SBUF-port contention with VectorE),
# gpsimd.partition_broadcast (4.4us/offset + doubles DVE op time), DMA
# row-replication (bandwidth-bound), fp32 matmuls (2x slower than fp16),
# 1024-wide fp16 moving operands (ISA-rejected).
import numpy as np

EPS = 1e-5
KK = 7
C = 128
H = 64
W = 64
B = 4
HH = 32              # output rows per core
XROW = 70            # padded row stride: 3 | 64 | 3
NH = HH + 6          # rows incl. 3-row halos
XPAD = 4             # edge guard (even: preserves dj parity)
NXF = NH * XROW + 2 * XPAD   # 2668 x columns per core
QOFF = XPAD + 3 * XROW       # start of own rows in guarded coords (214)
P = HH * W           # 2048 dense output pixels
MMCH = 512           # matmul moving chunk (= PSUM bank, fp32)
# process xa-aligned offsets (dj odd -> even base) first: the shifted twin
# xb is built on-device and only needed once the second group starts
OFFS = ([o for o in range(49) if o % 7 in (0, 2, 4, 6)]
        + [o for o in range(49) if o % 7 in (1, 3, 5)])

_STATE = {}


def _build():
    import concourse.tile as tile
    from concourse import bacc, mybir

    f32 = mybir.dt.float32
    f16 = mybir.dt.float16
    nc = bacc.Bacc("TRN2", target_bir_lowering=False, debug=False)

    xa_d = nc.dram_tensor("xa", [C, NXF], f16, kind="ExternalInput").ap()
    w1sT_d = nc.dram_tensor("w1sT", [C, 32], f16, kind="ExternalInput").ap()
    b1f_d = nc.dram_tensor("b1f", [32, 1], f32, kind="ExternalInput").ap()
    w2bc_d = nc.dram_tensor("w2bc", [32, 49 * C], f16, kind="ExternalInput").ap()
    b2bc_d = nc.dram_tensor("b2bc", [C, 49], f32, kind="ExternalInput").ap()
    out_d = nc.dram_tensor("out", [C, P], f16, kind="ExternalOutput").ap()

    with tile.TileContext(nc) as tc:
        with (
            tc.tile_pool(name="consts", bufs=1) as cpool,
            tc.tile_pool(name="bcs", bufs=6) as spool,
            tc.tile_pool(name="prod", bufs=4) as ppool,
            tc.tile_pool(name="pbc", bufs=4, space="PSUM") as pbc,
        ):
            # spread input DMAs across engines -> parallel hardware queues,
            # ordered so each consumer's gate arrives as late as it is needed
            HP = P // 2
            # xa in 4 row-chunks, alternating queues; gen chunk q gates on
            # the minimal rows it reads
            w1sT = cpool.tile([C, 32], f16, tag="w1")
            nc.sync.dma_start(w1sT[:], w1sT_d)
            xa = cpool.tile([C, NXF], f16, tag="xa")
            # first chunk (rows 0-12, gates gen mm 0 and offset 0) is split
            # across all three queues; the rest alternates sync/scalar
            c0c = XPAD + 12 * XROW
            for q in range(3):
                eng = (nc.sync, nc.scalar, nc.gpsimd)[q]
                lo = q * (c0c // 3) // 2 * 2
                hi = (q + 1) * (c0c // 3) // 2 * 2 if q < 2 else c0c
                eng.dma_start(xa[:, lo:hi], xa_d[:, lo:hi])
            cuts = (c0c, XPAD + 20 * XROW, XPAD + 28 * XROW, NXF)
            for q in range(3):
                eng = nc.sync if q % 2 == 0 else nc.scalar
                eng.dma_start(xa[:, cuts[q]:cuts[q + 1]],
                              xa_d[:, cuts[q]:cuts[q + 1]])
            w2bc = cpool.tile([32, 49 * C], f16, tag="w2bc")
            nc.sync.dma_start(w2bc[:, :8 * C], w2bc_d[:, :8 * C])
            # delay the bulk w2bc transfer until xa has landed: it is only
            # consumed from ~offset 8 (~40us in), and issuing it immediately
            # steals DMA bandwidth from the critical xa/gen path
            nc.gpsimd.tensor_copy(w2bc[0:1, 8 * C:8 * C + 2],
                                  xa[0:1, NXF - 2:NXF])
            nc.gpsimd.dma_start(w2bc[:, 8 * C:], w2bc_d[:, 8 * C:])
            b1f = cpool.tile([32, 1], f32, tag="b1")
            nc.sync.dma_start(b1f[:], b1f_d)
            b2bc = cpool.tile([C, 49], f32, tag="b2bc")
            nc.sync.dma_start(b2bc[:], b2bc_d)
            # twin of xa shifted by one element (keeps odd-dj reads 4B-aligned)
            xb = cpool.tile([C, NXF], f16, tag="xb")

            f_sb = cpool.tile([32, P], f16, tag="f")
            acc = cpool.tile([C, P], f16, tag="acc")

            # guarded-layout shifted views of x (3D: [128, 32 rows, 64 w])
            xar = xa[:, XPAD:XPAD + NH * XROW].rearrange("p (h w) -> p h w", w=XROW)
            xbr = xb[:, XPAD:XPAD + NH * XROW].rearrange("p (h w) -> p h w", w=XROW)

            # ---- kernel-feature generation: f = relu(w1s^T @ x + b1f) ----
            f_ps = pbc.tile([32, HP], f32, tag="bc")
            for (r, nr) in ((3, 4), (7, 4), (11, 8)):
                cl, ch = (r - 3) * 64, (r - 3 + nr) * 64
                nc.tensor.matmul(
                    f_ps[:, cl:ch], w1sT[:], xar[:, r:r + nr, 3:67],
                    start=True, stop=True,
                )
                nc.scalar.activation(
                    f_sb[:, cl:ch], f_ps[:, cl:ch],
                    mybir.ActivationFunctionType.Relu, bias=b1f[:],
                )
            f_ps = pbc.tile([32, HP], f32, tag="bc")
            for q in range(2):
                r = 19 + 8 * q
                nc.tensor.matmul(
                    f_ps[:, q * MMCH:(q + 1) * MMCH], w1sT[:],
                    xar[:, r:r + 8, 3:67], start=True, stop=True,
                )
            # hi-half f evac on VectorE's idle window via relu(x+b)=max(x+b,0),
            # keeping ScalarE's ramp free for the bcs chain
            z32 = cpool.tile([32, HP], f32, tag="z32")
            nc.vector.memset(z32[:], 0.0)
            nc.vector.scalar_tensor_tensor(
                f_sb[:, HP:], f_ps[:], b1f[:], z32[:],
                mybir.AluOpType.add, mybir.AluOpType.max,
            )

            # ---- involution accumulate over the 49 offsets ----
            HB = P // 2  # 1024: evac half (PSUM tile = 2 banks)
            for i, o in enumerate(OFFS):
                ip, jp = divmod(o, 7)
                di, dj = ip - 3, jp - 3
                bcs = spool.tile([C, P], f16, tag="bcs")
                for h2 in range(2):
                    bc = pbc.tile([C, HB], f32, tag="bc")
                    if i == 0 and h2 == 0:
                        pieces = ((0, 256), (256, 256), (512, 512))
                    elif i < 3 and h2 == 0:
                        pieces = ((0, 512), (512, 512))
                    else:
                        pieces = ((0, 1024),)
                    for (pl, pn) in pieces:
                        for ci in range(pl, pl + pn, MMCH):
                            cw = min(MMCH, pl + pn - ci)
                            nc.tensor.matmul(
                                bc[:, ci:ci + cw],
                                w2bc[:, i * C:(i + 1) * C],
                                f_sb[:, h2 * HB + ci:h2 * HB + ci + cw],
                                start=True, stop=True,
                            )
                        nc.scalar.activation(
                            bcs[:, h2 * HB + pl:h2 * HB + pl + pn],
                            bc[:, pl:pl + pn],
                            mybir.ActivationFunctionType.Identity,
                            bias=b2bc[:, o:o + 1],
                        )
                # shifted x view: rows di..di+32, cols 3+dj..67+dj of the
                # guarded layout; odd dj reads the 1-shifted twin for alignment
                r0 = 3 + di
                c0 = 3 + dj
                if c0 % 2 == 0:
                    xv = xar[:, r0:r0 + HH, c0:c0 + W]
                else:
                    xv = xbr[:, r0:r0 + HH, c0 - 1:c0 - 1 + W]
                bcsr = bcs.rearrange("p (h w) -> p h w", w=W)
                if i == 0:
                    # cascaded pieces track the evac pieces
                    accr = acc.rearrange("p (h w) -> p h w", w=W)
                    for (rl, rn) in ((0, 4), (4, 4), (8, 8), (16, 16)):
                        nc.vector.tensor_mul(
                            accr[:, rl:rl + rn], xv[:, rl:rl + rn],
                            bcsr[:, rl:rl + rn])
                elif i == 1:
                    prod = ppool.tile([C, P], f16, tag="prod")
                    prodr = prod.rearrange("p (h w) -> p h w", w=W)
                    nc.vector.tensor_mul(prodr[:, :8], xv[:, :8], bcsr[:, :8])
                    nc.vector.tensor_mul(prodr[:, 8:], xv[:, 8:], bcsr[:, 8:])
                    nc.vector.tensor_add(acc[:], acc[:], prod[:])
                elif i == 48:
                    prod = ppool.tile([C, P], f16, tag="prod")
                    prodr = prod.rearrange("p (h w) -> p h w", w=W)
                    nc.vector.tensor_mul(prodr, xv, bcsr)
                    TH = P // 3 // 2 * 2
                    nc.vector.tensor_add(acc[:, :TH], acc[:, :TH], prod[:, :TH])
                    nc.gpsimd.dma_start(out_d[:, :TH], acc[:, :TH])
                    nc.vector.tensor_add(acc[:, TH:2 * TH], acc[:, TH:2 * TH],
                                         prod[:, TH:2 * TH])
                    nc.scalar.dma_start(out_d[:, TH:2 * TH], acc[:, TH:2 * TH])
                    nc.vector.tensor_add(acc[:, 2 * TH:], acc[:, 2 * TH:],
                                         prod[:, 2 * TH:])
                    nc.sync.dma_start(out_d[:, 2 * TH:], acc[:, 2 * TH:])
                else:
                    prod = ppool.tile([C, P], f16, tag="prod")
                    prodr = prod.rearrange("p (h w) -> p h w", w=W)
                    nc.vector.tensor_mul(prodr, xv, bcsr)
                    nc.vector.tensor_add(acc[:], acc[:], prod[:])
                if i == 2:
                    # build the 1-shifted twin now: xa has fully landed, and
                    # the first twin-reading offset is ~26 offsets away
                    nc.vector.tensor_copy(xb[:, :NXF - 1], xa[:, 1:])

    nc.compile()
    return nc


def _get_nc():
    if "nc" not in _STATE:
        _STATE["nc"] = _build()
    return _STATE["nc"]


def _host_prep(x, w1, b1, bn_gamma, bn_beta, bn_mean, bn_var, w2, b2):
    x = np.asarray(x, dtype=np.float32)
    scale = np.asarray(bn_gamma) / np.sqrt(np.asarray(bn_var) + EPS)
    w1s = (np.asarray(w1) * scale[:, None]).astype(np.float32)
    b1f = (np.asarray(b1) * scale + np.asarray(bn_beta)
           - np.asarray(bn_mean) * scale).astype(np.float32)
    w1sT = np.ascontiguousarray(w1s.T).astype(np.float16)        # [128, 32]
    b1fc = np.ascontiguousarray(b1f[:, None])                    # [32, 1]
    w2f = np.asarray(w2, np.float32)                             # [49, 32]
    # W2BC[r, i*128 + c] = w2[OFFS[i], r]  (blocks in processing order)
    w2p = w2f.T[:, OFFS]                                     # [32, 49]
    w2bc = np.ascontiguousarray(
        np.broadcast_to(w2p[:, :, None], (32, 49, C)).reshape(32, 49 * C)
    ).astype(np.float16)
    b2bc = np.ascontiguousarray(
        np.broadcast_to(np.asarray(b2, np.float32), (C, 49))
    )

    x16 = x.astype(np.float16)
    in_maps = []
    for core in range(8):
        b, half = divmod(core, 2)
        h0 = HH * half
        xa = np.zeros((C, NXF), dtype=np.float16)
        lo = max(0, h0 - 3)
        hi = min(H, h0 + HH + 3)
        body = xa[:, XPAD:XPAD + NH * XROW].reshape(C, NH, XROW)
        body[:, lo - (h0 - 3):hi - (h0 - 3), 3:3 + W] = x16[b, :, lo:hi, :]
        in_maps.append({
            "xa": xa, "w1sT": w1sT, "b1f": b1fc,
            "w2bc": w2bc, "b2bc": b2bc,
        })
    return in_maps


def run(inputs: dict, trace: bool = False):
    from concourse.bass_utils import run_bass_kernel_spmd

    nc = _get_nc()
    in_maps = _host_prep(**inputs)
    res = run_bass_kernel_spmd(
        nc, in_maps, core_ids=list(range(8)), trace=trace,
    )
    out = np.zeros((B, C, H, W), dtype=np.float32)
    for core in range(8):
        b, half = divmod(core, 2)
        h0 = HH * half
        o = res.results[core]["out"].reshape(C, HH, W)
        out[b, :, h0:h0 + HH, :] = o.astype(np.float32)
    return out, res


def kernel(**inputs) -> np.ndarray:
    out, _ = run(inputs, trace=False)
    return out


# revision 45
# speedup vs baseline: 1.0088x; 1.0088x over previous
# Involution2d (K=7) Trainium2 kernel — 8-core SPMD, batch+spatial sharding.
#
# Sharding: 8 cores = (batch b in 0..3) x (H-half in 0..1). Each core computes
# a [128, 32, 64] output block. fp16 data path (rel-err budget 2e-2; lands
# ~2.4e-3).
#
# Per-core pipeline, software-pipelined across four engines:
#   1. f = relu(w1s^T @ x + b1f)            TensorE + ScalarE   [32, 2048]
#   2. per offset o (49 total):
#      bc_o  = W2BC_o @ f                   TensorE (K=32)      [128, 2048] PSUM
#        W2BC_o = w2 row o replicated into 128 columns (host-precomputed), so
#        the per-pixel kernel value is generated ALREADY BROADCAST over the
#        128 channel partitions; the [49, P] kernel map is never materialized
#        and there is no per-offset DMA.
#      bcs_o = bc_o + b2[o]                 ScalarE PSUM->SBUF fp16
#      prod  = bcs_o * x_shift(o)           VectorE fp16 2x_1P mode
#      acc  += prod                         VectorE fp16 2x_1P mode
#
# The 98-op VectorE stream (~1.14us per [128,2048] tensor_tensor) is the
# bottleneck and runs gap-free; TensorE (4 K=32 matmuls/offset) and ScalarE
# (PSUM evacuation + bias) hide underneath it.
#
# Layout tricks:
# - x lives in a guarded stride-70 row layout (3 zero cols per row edge,
#   3 halo rows top/bottom, zero-filled) so every shifted read is exact zero
#   padding — no W-edge mask, no wrap garbage. Shifted reads are 3D APs
#   [128, 32, 64] with row stride 70.
# - A twin copy of x offset by one element (built on-device by VectorE
#   during the DMA prologue) keeps odd-dj offsets 4-byte aligned — the DVE
#   2x_1P perf mode requires 4B-aligned stride-1 operands.
# - Offsets are processed xa-aligned-first so the twin is not needed until
#   ~60us into the stream.
# - acc/bcs/prod/f are dense stride-1 tiles; PSUM chunks are bank-aligned.
# - Input DMAs are split across the sync/scalar/gpsimd hardware queues and
#   ordered by when each consumer needs the data; the output is written in
#   thirds overlapped with the final accumulation.
#
# Rejected alternatives (measured): involution-as-banded-matmul (the skew/
# diagonal Kmat build has no fast primitive on TRN2), GpSimd elementwise
# offload (fp16 and fp32 both slower + SBUF-port contention with VectorE),
# gpsimd.partition_broadcast (4.4us/offset + doubles DVE op time), DMA
# row-replication (bandwidth-bound), fp32 matmuls (2x slower than fp16),
# 1024-wide fp16 moving operands (ISA-rejected).
import numpy as np

EPS = 1e-5
KK = 7
C = 128
H = 64
W = 64
B = 4
HH = 32              # output rows per core
XROW = 70            # padded row stride: 3 | 64 | 3
NH = HH + 6          # rows incl. 3-row halos
XPAD = 4             # edge guard (even: preserves dj parity)
NXF = NH * XROW + 2 * XPAD   # 2668 x columns per core
QOFF = XPAD + 3 * XROW       # start of own rows in guarded coords (214)
P = HH * W           # 2048 dense output pixels
MMCH = 512           # matmul moving chunk (= PSUM bank, fp32)
# process xa-aligned offsets (dj odd -> even base) first: the shifted twin
# xb is built on-device and only needed once the second group starts
OFFS = ([o for o in range(49) if o % 7 in (0, 2, 4, 6)]
        + [o for o in range(49) if o % 7 in (1, 3, 5)])

_STATE = {}


def _build():
    import concourse.tile as tile
    from concourse import bacc, mybir

    f32 = mybir.dt.float32
    f16 = mybir.dt.float16
    nc = bacc.Bacc("TRN2", target_bir_lowering=False, debug=False)

    xa_d = nc.dram_tensor("xa", [C, NXF], f16, kind="ExternalInput").ap()
    w1sT_d = nc.dram_tensor("w1sT", [C, 32], f16, kind="ExternalInput").ap()
    b1f_d = nc.dram_tensor("b1f", [32, 1], f32, kind="ExternalInput").ap()
    w2bc_d = nc.dram_tensor("w2bc", [32, 49 * C], f16, kind="ExternalInput").ap()
    b2bc_d = nc.dram_tensor("b2bc", [C, 49], f32, kind="ExternalInput").ap()
    out_d = nc.dram_tensor("out", [C, P], f16, kind="ExternalOutput").ap()

    with tile.TileContext(nc) as tc:
        with (
            tc.tile_pool(name="consts", bufs=1) as cpool,
            tc.tile_pool(name="bcs", bufs=6) as spool,
            tc.tile_pool(name="prod", bufs=4) as ppool,
            tc.tile_pool(name="pbc", bufs=4, space="PSUM") as pbc,
        ):
            # spread input DMAs across engines -> parallel hardware queues,
            # ordered so each consumer's gate arrives as late as it is needed
            HP = P // 2
            # xa in 4 row-chunks, alternating queues; gen chunk q gates on
            # the minimal rows it reads
            w1sT = cpool.tile([C, 32], f16, tag="w1")
            nc.sync.dma_start(w1sT[:], w1sT_d)
            xa = cpool.tile([C, NXF], f16, tag="xa")
            # first chunk (rows 0-12, gates gen mm 0 and offset 0) is split
            # across all three queues; the rest alternates sync/scalar
            c0c = XPAD + 12 * XROW
            for q in range(3):
                eng = (nc.sync, nc.scalar, nc.gpsimd)[q]
                lo = q * (c0c // 3) // 2 * 2
                hi = (q + 1) * (c0c // 3) // 2 * 2 if q < 2 else c0c
                eng.dma_start(xa[:, lo:hi], xa_d[:, lo:hi])
            cuts = (c0c, XPAD + 20 * XROW, XPAD + 28 * XROW, NXF)
            for q in range(3):
                eng = nc.sync if q % 2 == 0 else nc.scalar
                eng.dma_start(xa[:, cuts[q]:cuts[q + 1]],
                              xa_d[:, cuts[q]:cuts[q + 1]])
            w2bc = cpool.tile([32, 49 * C], f16, tag="w2bc")
            nc.sync.dma_start(w2bc[:, :8 * C], w2bc_d[:, :8 * C])
            # delay the bulk w2bc transfer until xa has landed: it is only
            # consumed from ~offset 8 (~40us in), and issuing it immediately
            # steals DMA bandwidth from the critical xa/gen path
            nc.gpsimd.tensor_copy(w2bc[0:1, 8 * C:8 * C + 2],
                                  xa[0:1, NXF - 2:NXF])
            nc.gpsimd.dma_start(w2bc[:, 8 * C:], w2bc_d[:, 8 * C:])
            b1f = cpool.tile([32, 1], f32, tag="b1")
            nc.sync.dma_start(b1f[:], b1f_d)
            b2bc = cpool.tile([C, 49], f32, tag="b2bc")
            nc.sync.dma_start(b2bc[:], b2bc_d)
            # twin of xa shifted by one element (keeps odd-dj reads 4B-aligned)
            xb = cpool.tile([C, NXF], f16, tag="xb")

            f_sb = cpool.tile([32, P], f16, tag="f")
            acc = cpool.tile([C, P], f16, tag="acc")

            # guarded-layout shifted views of x (3D: [128, 32 rows, 64 w])
            xar = xa[:, XPAD:XPAD + NH * XROW].rearrange("p (h w) -> p h w", w=XROW)
            xbr = xb[:, XPAD:XPAD + NH * XROW].rearrange("p (h w) -> p h w", w=XROW)

            # ---- kernel-feature generation: f = relu(w1s^T @ x + b1f) ----
            for hg in range(2):
                f_ps = pbc.tile([32, HP], f32, tag="bc")
                for q in range(2):
                    r = 3 + 8 * (2 * hg + q)
                    nc.tensor.matmul(
                        f_ps[:, q * MMCH:(q + 1) * MMCH], w1sT[:],
                        xar[:, r:r + 8, 3:67], start=True, stop=True,
                    )
                    if hg == 0:
                        nc.scalar.activation(
                            f_sb[:, q * MMCH:(q + 1) * MMCH],
                            f_ps[:, q * MMCH:(q + 1) * MMCH],
                            mybir.ActivationFunctionType.Relu, bias=b1f[:],
                        )
                if hg == 1:
                    nc.scalar.activation(
                        f_sb[:, HP:], f_ps[:],
                        mybir.ActivationFunctionType.Relu, bias=b1f[:],
                    )

            # ---- involution accumulate over the 49 offsets ----
            HB = P // 2  # 1024: evac half (PSUM tile = 2 banks)
            for i, o in enumerate(OFFS):
                ip, jp = divmod(o, 7)
                di, dj = ip - 3, jp - 3
                bcs = spool.tile([C, P], f16, tag="bcs")
                for h2 in range(2):
                    bc = pbc.tile([C, HB], f32, tag="bc")
                    for ci in range(HB // MMCH):
                        c0 = h2 * HB + ci * MMCH
                        nc.tensor.matmul(
                            bc[:, ci * MMCH:(ci + 1) * MMCH],
                            w2bc[:, i * C:(i + 1) * C],
                            f_sb[:, c0:c0 + MMCH],
                            start=True, stop=True,
                        )
                    if i < 2 and h2 == 0:
                        for ci in range(2):
                            nc.scalar.activation(
                                bcs[:, ci * MMCH:(ci + 1) * MMCH],
                                bc[:, ci * MMCH:(ci + 1) * MMCH],
                                mybir.ActivationFunctionType.Identity,
                                bias=b2bc[:, o:o + 1],
                            )
                    else:
                        nc.scalar.activation(
                            bcs[:, h2 * HB:(h2 + 1) * HB], bc[:],
                            mybir.ActivationFunctionType.Identity,
                            bias=b2bc[:, o:o + 1],
                        )
                # shifted x view: rows di..di+32, cols 3+dj..67+dj of the
                # guarded layout; odd dj reads the 1-shifted twin for alignment
                r0 = 3 + di
                c0 = 3 + dj
                if c0 % 2 == 0:
                    xv = xar[:, r0:r0 + HH, c0:c0 + W]
                else:
                    xv = xbr[:, r0:r0 + HH, c0 - 1:c0 - 1 + W]
                bcsr = bcs.rearrange("p (h w) -> p h w", w=W)
                if i == 0:
                    # quartered so each piece starts right after its evac
                    accr = acc.rearrange("p (h w) -> p h w", w=W)
                    nc.vector.tensor_mul(accr[:, :8], xv[:, :8], bcsr[:, :8])
                    nc.vector.tensor_mul(accr[:, 8:16], xv[:, 8:16],
                                         bcsr[:, 8:16])
                    nc.vector.tensor_mul(accr[:, 16:], xv[:, 16:], bcsr[:, 16:])
                elif i == 1:
                    prod = ppool.tile([C, P], f16, tag="prod")
                    prodr = prod.rearrange("p (h w) -> p h w", w=W)
                    nc.vector.tensor_mul(prodr[:, :8], xv[:, :8], bcsr[:, :8])
                    nc.vector.tensor_mul(prodr[:, 8:], xv[:, 8:], bcsr[:, 8:])
                    nc.vector.tensor_add(acc[:], acc[:], prod[:])
                elif i == 48:
                    prod = ppool.tile([C, P], f16, tag="prod")
                    prodr = prod.rearrange("p (h w) -> p h w", w=W)
                    nc.vector.tensor_mul(prodr, xv, bcsr)
                    TH = P // 3 // 2 * 2
                    nc.vector.tensor_add(acc[:, :TH], acc[:, :TH], prod[:, :TH])
                    nc.gpsimd.dma_start(out_d[:, :TH], acc[:, :TH])
                    nc.vector.tensor_add(acc[:, TH:2 * TH], acc[:, TH:2 * TH],
                                         prod[:, TH:2 * TH])
                    nc.scalar.dma_start(out_d[:, TH:2 * TH], acc[:, TH:2 * TH])
                    nc.vector.tensor_add(acc[:, 2 * TH:], acc[:, 2 * TH:],
                                         prod[:, 2 * TH:])
                    nc.sync.dma_start(out_d[:, 2 * TH:], acc[:, 2 * TH:])
                else:
                    prod = ppool.tile([C, P], f16, tag="prod")
                    prodr = prod.rearrange("p (h w) -> p h w", w=W)
                    nc.vector.tensor_mul(prodr, xv, bcsr)
                    nc.vector.tensor_add(acc[:], acc[:], prod[:])
                if i == 2:
                    # build the 1-shifted twin now: xa has fully landed, and
                    # the first twin-reading offset is ~26 offsets away
                    nc.vector.tensor_copy(xb[:, :NXF - 1], xa[:, 1:])

    nc.compile()
    return nc


def _get_nc():
    if "nc" not in _STATE:
        _STATE["nc"] = _build()
    return _STATE["nc"]


def _host_prep(x, w1, b1, bn_gamma, bn_beta, bn_mean, bn_var, w2, b2):
    x = np.asarray(x, dtype=np.float32)
    scale = np.asarray(bn_gamma) / np.sqrt(np.asarray(bn_var) + EPS)
    w1s = (np.asarray(w1) * scale[:, None]).astype(np.float32)
    b1f = (np.asarray(b1) * scale + np.asarray(bn_beta)
           - np.asarray(bn_mean) * scale).astype(np.float32)
    w1sT = np.ascontiguousarray(w1s.T).astype(np.float16)        # [128, 32]
    b1fc = np.ascontiguousarray(b1f[:, None])                    # [32, 1]
    w2f = np.asarray(w2, np.float32)                             # [49, 32]
    # W2BC[r, i*128 + c] = w2[OFFS[i], r]  (blocks in processing order)
    w2p = w2f.T[:, OFFS]                                     # [32, 49]
    w2bc = np.ascontiguousarray(
        np.broadcast_to(w2p[:, :, None], (32, 49, C)).reshape(32, 49 * C)
    ).astype(np.float16)
    b2bc = np.ascontiguousarray(
        np.broadcast_to(np.asarray(b2, np.float32), (C, 49))
    )

    x16 = x.astype(np.float16)
    in_maps = []
    for core in range(8):
        b, half = divmod(core, 2)
        h0 = HH * half
        xa = np.zeros((C, NXF), dtype=np.float16)
        lo = max(0, h0 - 3)
        hi = min(H, h0 + HH + 3)
        body = xa[:, XPAD:XPAD + NH * XROW].reshape(C, NH, XROW)
        body[:, lo - (h0 - 3):hi - (h0 - 3), 3:3 + W] = x16[b, :, lo:hi, :]
        in_maps.append({
            "xa": xa, "w1sT": w1sT, "b1f": b1fc,
            "w2bc": w2bc, "b2bc": b2bc,
        })
    return in_maps


def run(inputs: dict, trace: bool = False):
    from concourse.bass_utils import run_bass_kernel_spmd

    nc = _get_nc()
    in_maps = _host_prep(**inputs)
    res = run_bass_kernel_spmd(
        nc, in_maps, core_ids=list(range(8)), trace=trace,
    )
    out = np.zeros((B, C, H, W), dtype=np.float32)
    for core in range(8):
        b, half = divmod(core, 2)
        h0 = HH * half
        o = res.results[core]["out"].reshape(C, HH, W)
        out[b, :, h0:h0 + HH, :] = o.astype(np.float32)
    return out, res


def kernel(**inputs) -> np.ndarray:
    out, _ = run(inputs, trace=False)
    return out
